# revision 2
# baseline (speedup 1.0000x reference)
"""GatingAttentionLayerWsa on 8 TRN2 NeuronCores — v2.

Shapes: B=4, S=L=2048, E=512, H=8, D=64.

Sharding: core c = (batch b=c//2, query-half c%2). Each core holds the FULL
K/V for its batch (projections duplicated across the pair) and only its half
of the query rows (L/2=1024). No collective; the host concatenates.

v2 changes vs v1 (365us baseline):
 - ACT runs ONLY Exp/Ln (both live in the natural_log_exp table): the row
   std is r = exp(-0.5*ln(pv' + S*eps) + 0.5*ln(S)) = 1/sqrt(var+eps), so
   no Sqrt table thrash and no ACT copies anywhere.
 - Covariance + kT transposes interleave with the K projection (lag-one
   group software pipeline), removing the serial cov/kT phase.
 - Paired-head tiles: kT2/qT2 hold head pairs stacked on 128 partitions
   (odd head at base 64 — matmul lhsT/rhs bases match; tile_position
   (64,0) is legal for 64-row stationaries). Halves transpose count and
   lets stats run on 128-partition block-diagonal cc2bd stationaries.
 - cov runs on UNSCALED K^TK (1/S folded into the ln/exp constants).
 - Attention inner loop emits QK(sc+1) before PV(sc) so PE streams while
   ACT exps chunk sc (pz double-buffered).
 - Out-projection in bf16 (oTp bf16, Wo bf16).
 - k_nat/v_sb ones columns memset once (outside the timing rep loop).
"""
import math
import sys
from contextlib import ExitStack

import numpy as np

try:
    import concourse.bass as bass  # noqa: F401
except ImportError:  # pragma: no cover
    sys.path.insert(0, "/opt/trn_rl_repo")

import concourse.bacc as bacc
import concourse.mybir as mybir
import concourse.tile as tile
from concourse import masks
from concourse.bass_utils import run_bass_kernel_spmd

B, S, E, H, D = 4, 2048, 512, 8, 64
LH = S // 2        # 1024 query rows per core
N_CORES = 8
D1 = D + 1         # head slot width incl ones column
KW = H * D1        # 520: k/v natural width with interleaved ones
NSC = S // 128     # 16 chunks of 128 along S
NQC = LH // 128    # 8 chunks of 128 along the L-half
NEC = E // 128     # 4 chunks of 128 along E
NT = 4             # head pairs
F32 = mybir.dt.float32
F32R = mybir.dt.float32r
BF16 = mybir.dt.bfloat16
AF = mybir.ActivationFunctionType

_CACHE = {}


def _pin_act_tables():
    """Restrict Exp/Ln resolution to the shared natural_log_exp table.

    The table-load fixpoint picks the FIRST table containing each
    function; Exp would land in exp_and_others and Ln in natural_log,
    thrashing a 1.3us table load at every stats block. Filtering Exp/Ln
    from every other table (order and thus act_func_set ids preserved)
    yields exactly one load. Returns a context manager."""
    import contextlib
    from concourse import hw_specs

    @contextlib.contextmanager
    def ctx():
        orig = bacc.get_activation_tables

        def patched(arch):
            tabs = dict(orig(arch))
            out = {}
            for name, fns in tabs.items():
                if name != "natural_log_exp_and_others":
                    fns = {f for f in fns
                           if f not in (AF.Exp, AF.Ln)}
                out[name] = fns
            return out

        bacc.get_activation_tables = patched
        try:
            yield
        finally:
            bacc.get_activation_tables = orig
    return ctx()




def _build(reps=1, debug=False):
    nc = bacc.Bacc("TRN2", target_bir_lowering=False, debug=False,
                   num_devices=N_CORES)
    dbg = {}
    if debug:
        for nm, shape, dt in [
                ("d_knat0", [128, KW], BF16), ("d_vsb0", [128, KW], BF16),
                ("d_kT2_0", [128, S], BF16), ("d_qT2_0", [128, LH], BF16),
                ("d_cc2_0", [128, 128], BF16), ("d_lnr0", [1, LH], F32),
                ("d_rr0", [1, LH], BF16), ("d_rr1", [1, LH], BF16),
                ("d_psb0", [128, LH], BF16), ("d_oT0", [128, LH], BF16),
                ("d_pc4", [D1, 8 * D1], F32)]:
            dbg[nm] = nc.dram_tensor(nm, shape, dt,
                                     kind="ExternalOutput").ap()
    src_q = nc.dram_tensor("src_q", [LH, E], F32, kind="ExternalInput").ap()
    src_k = nc.dram_tensor("src_k", [S, E], F32, kind="ExternalInput").ap()
    src_v = nc.dram_tensor("src_v", [S, E], F32, kind="ExternalInput").ap()
    wq = nc.dram_tensor("wq", [E, E], F32, kind="ExternalInput").ap()
    wk = nc.dram_tensor("wk", [E, E], F32, kind="ExternalInput").ap()
    wv = nc.dram_tensor("wv", [E, E], F32, kind="ExternalInput").ap()
    wo = nc.dram_tensor("wo", [E, E], F32, kind="ExternalInput").ap()
    bq = nc.dram_tensor("bq", [1, E], F32, kind="ExternalInput").ap()
    bk = nc.dram_tensor("bk", [1, E], F32, kind="ExternalInput").ap()
    bv = nc.dram_tensor("bv", [1, E], F32, kind="ExternalInput").ap()
    bo = nc.dram_tensor("bo", [1, E], F32, kind="ExternalInput").ap()
    out = nc.dram_tensor("out", [LH, E], F32, kind="ExternalOutput").ap()

    with tile.TileContext(nc) as tc, ExitStack() as X:
        sb = X.enter_context(tc.tile_pool(name="sb", bufs=1))

        # ---- constants + persistent tiles (once, outside the rep loop) ----
        identf = sb.tile([128, 128], F32)
        masks.make_identity(nc, identf[:])
        identr_t = sb.tile([128, 128], F32R)
        nc.vector.tensor_copy(identr_t[:], identf[:])
        identr = identr_t[:]
        identb = sb.tile([128, 128], BF16)
        nc.vector.tensor_copy(identb[:], identf[:])
        ones64b = sb.tile([64, 1], BF16)
        nc.gpsimd.memset(ones64b[:], 1.0)
        epsb = sb.tile([1, 1], F32)
        nc.gpsimd.memset(epsb[:], float(S) * 1e-6)  # ln bias: S*eps
        lnsb = sb.tile([1, 1], F32)
        nc.gpsimd.memset(lnsb[:], 0.5 * math.log(S))  # exp bias: +0.5*ln(S)

        # persistent attention-side tensors (ones cols written once)
        k_nat = [sb.tile([128, KW], BF16, name=f"kn{sc}") for sc in range(NSC)]
        v_sb = [sb.tile([128, KW], BF16, name=f"vn{sc}") for sc in range(NSC)]
        for tl in k_nat + v_sb:
            d3 = tl[:].rearrange("p (h w) -> p h w", h=H)
            nc.gpsimd.memset(d3[:, :, D:D1], 1.0)
        kT2 = [sb.tile([128, S], BF16, name=f"kT2_{t}") for t in range(NT)]
        qT2 = [sb.tile([128, LH], BF16, name=f"qT2_{t}") for t in range(NT)]
        cc2bd = [sb.tile([128, 128], BF16, name=f"cc2_{t}") for t in range(NT)]
        for tl in cc2bd:
            nc.gpsimd.memset(tl[:], 0.0)

        def body():
            with ExitStack() as XR:
                pr = XR.enter_context(tc.tile_pool(name="pr", bufs=1))

                def load_w(name, src, dt=BF16):
                    """Weights: f32 DMA stage + Pool cast to bf16."""
                    ts = []
                    for e in range(NEC):
                        tf = pr.tile([128, E], F32, name=f"{name}f{e}",
                                     tag="wstage", bufs=2)
                        nc.sync.dma_start(tf[:],
                                          src[e * 128:(e + 1) * 128, :])
                        t = pr.tile([128, E], dt, name=f"{name}{e}",
                                    tag=f"{name}{e}")
                        nc.gpsimd.tensor_copy(t[:], tf[:])
                        ts.append(t)
                    return ts

                def bcast_bias(name, src):
                    row = pr.tile([1, E], F32, name=f"{name}_row",
                                  tag=f"{name}_row")
                    nc.sync.dma_start(row[:], src[:])
                    full = pr.tile([128, E], F32, name=f"{name}_b",
                                   tag=f"{name}_b")
                    nc.gpsimd.partition_broadcast(full[:], row[:])
                    return full

                def emit_loads(src, nrows, who):
                    nats = []
                    for sc in range(nrows // 128):
                        nat = pr.tile([128, E], F32R, name=f"nat{who}{sc}",
                                      tag="nat", bufs=6)
                        nc.sync.dma_start(
                            nat[:],
                            src[sc * 128:(sc + 1) * 128, :].bitcast(F32R))
                        nats.append(nat)
                    return nats

                # ---- DMA emission order: q first (feeds early PE work),
                # then wk+bk, k, wq+bq, v, wv+bv, wo+bo ----
                q_nats = emit_loads(src_q, LH, "q")
                wk_t = load_w("wk", wk)
                bkb = bcast_bias("bk", bk)
                k_nats = emit_loads(src_k, S, "k")
                wq_t = load_w("wq", wq)
                bqc = []
                for t in range(NT):
                    c = pr.tile([128, 1], F32, name=f"bqc{t}", tag=f"bqc{t}")
                    nc.sync.dma_start(
                        c[:],
                        bq[0:1, t * 128:(t + 1) * 128].rearrange("a b -> b a"))
                    bqc.append(c)
                v_nats = emit_loads(src_v, S, "v")
                wv_t = load_w("wv", wv)
                bvb = bcast_bias("bv", bv)
                wo_t = load_w("wo", wo)
                bob = bcast_bias("bo", bo)

                scr = None  # attention-phase PSUM pool, opened post-K

                def transpose_group(nats, g, who, psum, ptag, pbufs):
                    """PE-transpose 4 f32r nat chunks into bf16 [128,512] X^T
                    slices per e-chunk."""
                    sg = []
                    for e in range(NEC):
                        pt = psum.tile([128, 512], F32R,
                                       name=f"pt{who}{g}_{e}",
                                       tag=ptag, bufs=pbufs)
                        for i in range(4):
                            nc.tensor.transpose(
                                pt[:, i * 128:(i + 1) * 128],
                                nats[g * 4 + i][:, e * 128:(e + 1) * 128],
                                identr)
                        sgt = pr.tile(
                            [128, 512], BF16, name=f"sg{who}{g}_{e}",
                            tag=(f"qsg{g}_{e}" if who == "q" else f"sg{e}"),
                            bufs=(1 if who == "q" else 2))
                        nc.vector.tensor_copy(sgt[:], pt[:].bitcast(F32))
                        sg.append(sgt)
                    return sg

                def proj_group(sg, g, w_t, bias_b, dst, who, psum, ptag,
                               pbufs):
                    """dst[sc] [128, KW] bf16 = [x@W + b | ones] for the 4
                    chunks of group g (ones cols pre-set, untouched here)."""
                    for i in range(4):
                        sc = g * 4 + i
                        pp = psum.tile([128, E], F32, name=f"pp{who}{sc}",
                                       tag=ptag, bufs=pbufs)
                        for e in range(NEC):
                            nc.tensor.matmul(
                                pp[:], sg[e][:, i * 128:(i + 1) * 128],
                                w_t[e][:], start=(e == 0), stop=(e == NEC - 1))
                        d3 = dst[sc][:].rearrange("p (h w) -> p h w", h=H)
                        nc.vector.tensor_add(
                            d3[:, :, 0:D],
                            pp[:].rearrange("p (h w) -> p h w", h=H),
                            bias_b[:].rearrange("p (h w) -> p h w", h=H))

                # ---- q transposes first: fills PE while K streams in ----
                kscr_cm = tc.tile_pool(name="kscr", bufs=1, space="PSUM")
                kscr = kscr_cm.__enter__()
                qsg = [transpose_group(q_nats, g, "q", kscr, "ks", 4)
                       for g in range(2)]

                # ---- K phase: proj group g ++ (cov + kT2) of group g-1 ----
                with tc.tile_pool(name="pck", bufs=1, space="PSUM") as pck:
                    # -- per group: per-pair kT transposes into [128,512]
                    # psum slices, then one DVE copy each (transposes are
                    # singleton matmul groups: safe to share a bank) --
                    def kt_group(g):
                        ptks = [pck.tile([128, 1024], BF16,
                                         name=f"ptk{g}_{i}", tag=f"ptk{i}",
                                         bufs=1) for i in range(2)]
                        for i in range(4):
                            sc = g * 4 + i
                            for h in range(H):
                                t, hh = h // 2, h % 2
                                cols = slice((t % 2) * 512 + i * 128,
                                             (t % 2) * 512 + (i + 1) * 128)
                                nc.tensor.transpose(
                                    ptks[t // 2][hh * 64:hh * 64 + 64, cols],
                                    k_nat[sc][:, h * D1:h * D1 + D], identb)
                        for t in range(NT):
                            nc.vector.tensor_copy(
                                kT2[t][:, g * 512:(g + 1) * 512],
                                ptks[t // 2][:, (t % 2) * 512:
                                             (t % 2 + 1) * 512])

                    for g in range(4):
                        sgk = transpose_group(k_nats, g, "k", kscr, "ks", 4)
                        proj_group(sgk, g, wk_t, bkb, k_nat, "k", kscr,
                                   "ks", 4)
                        if g >= 1:
                            kt_group(g - 1)
                    kt_group(3)

                    # -- covariance: one sequential 16-chunk chain per head,
                    # alternating psum banks (interleaved chains in a shared
                    # bank corrupt each other: start resets the whole bank) --
                    pc_t = [pck.tile([D1, D1], F32, name=f"pc{h}",
                                     tag=f"pc{h % 2}", bufs=1)
                            for h in range(H)]
                    for h in range(H):
                        ka = slice(h * D1, (h + 1) * D1)
                        for sc in range(NSC):
                            nc.tensor.matmul(pc_t[h][:], k_nat[sc][:, ka],
                                             k_nat[sc][:, ka],
                                             start=(sc == 0),
                                             stop=(sc == NSC - 1))

                    # ---- cc2bd blocks: cc' = K^TK - (Sk̄)k̄^T (unscaled) ----
                    if debug:
                        for h in range(H):
                            pcs = pr.tile([D1, D1], F32, name=f"pcs{h}",
                                          tag="pcs", bufs=2)
                            nc.vector.tensor_copy(pcs[:], pc_t[h][:])
                            nc.sync.dma_start(
                                dbg["d_pc4"][:, h * D1:(h + 1) * D1], pcs[:])
                    if True:
                        for h in range(H):
                            pc = pc_t[h][:]
                            pcrow = pr.tile([1, D1], BF16, name=f"pcr{h}",
                                            tag="pcr", bufs=2)
                            nc.vector.tensor_copy(pcrow[:], pc[D:D1, :])
                            kmrow = pr.tile([1, D1], BF16, name=f"kmr{h}",
                                            tag="kmr", bufs=2)
                            nc.vector.tensor_scalar_mul(kmrow[:], pc[D:D1, :],
                                                        1.0 / S)
                            ccp = pr.tile([D, D], BF16, name=f"ccp{h}",
                                          tag="ccp", bufs=2)
                            nc.vector.tensor_copy(ccp[:], pc[0:D, 0:D])
                            oo = pck.tile([D, D], F32, name=f"oo{h}",
                                          tag=f"ptk{h % 2}", bufs=1)
                            # (reuses a transpose bank; transposes done)
                            nc.tensor.matmul(oo[:], pcrow[:, 0:D],
                                             kmrow[:, 0:D], start=True,
                                             stop=True)
                            rh = (h % 2) * 64
                            nc.vector.tensor_sub(
                                cc2bd[h // 2][rh:rh + 64, rh:rh + 64],
                                ccp[:], oo[:])

                kscr_cm.__exit__(None, None, None)
                scr_cm = tc.tile_pool(name="scr", bufs=1, space="PSUM")
                scr = scr_cm.__enter__()

                # ---- interleaved Q proj / stats / V / attention ----
                oTp = [pr.tile([128, LH], BF16, name=f"oT{t}", tag=f"oT{t}")
                       for t in range(NT)]
                yac = [pr.tile([128, E], F32, name=f"yac{lc}", tag=f"yac{lc}")
                       for lc in range(NQC)]

                def qproj_stats(t):
                    """q pair projection into qT2[t], then r folded in.

                    r = exp(-0.5*ln(pv' + S*eps) + 0.5*ln(S)) applied to the
                    raw q; pv' = q^T cc' q uses the unscaled covariance. The
                    odd head's raw q is staged in a base-0 tile so every DVE
                    tensor-tensor keeps base-0 SBUF inputs.
                    """
                    qodd = pr.tile([64, LH], BF16, name=f"qodd{t}",
                                   tag="qodd", bufs=2)
                    for g in range(2):
                        js = slice(g * 512, (g + 1) * 512)
                        pp = scr.tile([128, 512], F32, name=f"ppq{t}{g}",
                                      tag="pp", bufs=2)
                        for e in range(NEC):
                            nc.tensor.matmul(
                                pp[:], wq_t[e][:, t * 128:(t + 1) * 128],
                                qsg[g][e][:], start=(e == 0),
                                stop=(e == NEC - 1))
                        nc.vector.tensor_scalar_add(qT2[t][0:64, js],
                                                    pp[0:64, :],
                                                    bqc[t][0:64, :])
                        nc.vector.tensor_scalar_add(qodd[:, js],
                                                    pp[64:128, :],
                                                    bqc[t][64:128, :])
                        # raw odd also lands in qT2[64:128] so the paired
                        # block-diag pu matmul sees it; the r-scale below
                        # overwrites it in place from the qodd staging copy
                        nc.vector.tensor_scalar_add(qT2[t][64:128, js],
                                                    pp[64:128, :],
                                                    bqc[t][64:128, :])
                    lnr = [pr.tile([1, LH], F32, name=f"lnr{t}{hh}",
                                   tag="lnr", bufs=3) for hh in range(2)]
                    for g in range(2):
                        js = slice(g * 512, (g + 1) * 512)
                        pu = scr.tile([128, 512], F32, name=f"pu{t}{g}",
                                      tag="pp", bufs=2)
                        nc.tensor.matmul(pu[:], cc2bd[t][:], qT2[t][:, js],
                                         start=True, stop=True)
                        for hh in range(2):
                            wb = pr.tile([64, 512], BF16, name=f"wb{t}{g}{hh}",
                                         tag="wb", bufs=2)
                            nc.vector.tensor_mul(
                                wb[:], pu[hh * 64:hh * 64 + 64, :],
                                qT2[t][0:64, js] if hh == 0 else qodd[:, js])
                            pv = scr.tile([1, 512], F32, name=f"pv{t}{g}{hh}",
                                          tag="pp", bufs=2)
                            nc.tensor.matmul(pv[:], ones64b[:], wb[:],
                                             start=True, stop=True)
                            nc.scalar.activation(lnr[hh][:, js], pv[:],
                                                 AF.Ln, bias=epsb[:])
                    for hh in range(2):
                        rrow = pr.tile([1, LH], BF16, name=f"rr{t}{hh}",
                                       tag="rrow", bufs=3)
                        nc.scalar.activation(rrow[:], lnr[hh][:], AF.Exp,
                                             bias=lnsb[:], scale=-0.5)
                        rb = pr.tile([64, LH], BF16, name=f"rb{t}{hh}",
                                     tag="rb", bufs=2)
                        nc.gpsimd.partition_broadcast(rb[:], rrow[:])
                        if hh == 0:
                            nc.vector.tensor_mul(qT2[t][0:64, :],
                                                 qT2[t][0:64, :], rb[:])
                        else:
                            nc.vector.tensor_mul(qT2[t][64:128, :],
                                                 qodd[:], rb[:])
                        if debug and t == 0:
                            nc.sync.dma_start(dbg[f"d_rr{hh}"][:], rrow[:])
                    if debug and t == 0:
                        nc.sync.dma_start(dbg["d_lnr0"][:], lnr[0][:])
                        nc.sync.dma_start(dbg["d_qT2_0"][:], qT2[0][:])

                def emit_v_group(g):
                    sgv = transpose_group(v_nats, g, "v", scr, "pp", 2)
                    proj_group(sgv, g, wv_t, bvb, v_sb, "v", scr, "pp", 2)

                with tc.tile_pool(name="pat", bufs=1, space="PSUM") as pat:
                    qproj_stats(0)
                    if debug:
                        nc.sync.dma_start(dbg["d_knat0"][:], k_nat[0][:])
                        nc.sync.dma_start(dbg["d_kT2_0"][:], kT2[0][:])
                        nc.sync.dma_start(dbg["d_cc2_0"][:], cc2bd[0][:])

                    for t in range(NT):
                        for hh in range(2):
                            h = 2 * t + hh
                            rh = hh * 64
                            if hh == 1 and t < NT - 1:
                                qproj_stats(t + 1)
                            po = pat.tile([D1, LH], F32, name=f"po{h}",
                                          tag="po", bufs=1)

                            def emit_qk(sc):
                                pz = pat.tile([128, LH], F32,
                                              name=f"pz{h}_{sc}", tag="pz",
                                              bufs=2)
                                for j in range(2):
                                    js = slice(j * 512, (j + 1) * 512)
                                    nc.tensor.matmul(
                                        pz[:, js],
                                        kT2[t][rh:rh + 64,
                                               sc * 128:(sc + 1) * 128],
                                        qT2[t][rh:rh + 64, js],
                                        start=True, stop=True)
                                psb = pr.tile([128, LH], BF16,
                                              name=f"psb{h}_{sc}", tag="psb",
                                              bufs=4)
                                nc.scalar.activation(psb[:], pz[:], AF.Exp)
                                return psb

                            psb_next = emit_qk(0)
                            for sc in range(NSC):
                                psb_cur = psb_next
                                if debug and h == 0 and sc == 0:
                                    nc.sync.dma_start(dbg["d_psb0"][:],
                                                      psb_cur[:])
                                if sc + 1 < NSC:
                                    psb_next = emit_qk(sc + 1)
                                if h == 0 and sc % 4 == 0:
                                    emit_v_group(sc // 4)
                                for j in range(2):
                                    js = slice(j * 512, (j + 1) * 512)
                                    nc.tensor.matmul(
                                        po[:, js],
                                        v_sb[sc][:, h * D1:(h + 1) * D1],
                                        psb_cur[:, js], start=(sc == 0),
                                        stop=(sc == NSC - 1))

                            rs = pr.tile([1, LH], F32, name=f"rs{h}",
                                         tag="brow", bufs=3)
                            nc.vector.tensor_copy(rs[:], po[D:D1, :])
                            ri = pr.tile([1, LH], F32, name=f"ri{h}",
                                         tag="brow", bufs=3)
                            nc.vector.reciprocal_approx_fast(ri[:], rs[:])
                            ib = pr.tile([64, LH], F32, name=f"ib{h}",
                                         tag="ib", bufs=2)
                            nc.gpsimd.partition_broadcast(ib[:], ri[:])
                            nc.vector.tensor_mul(
                                oTp[t][rh:rh + 64, :], po[0:D, :], ib[:])

                        if debug and t == 0:
                            nc.sync.dma_start(dbg["d_oT0"][:], oTp[0][:])
                            nc.sync.dma_start(dbg["d_vsb0"][:], v_sb[0][:])
                        # fold pair t's slice of the out-projection into
                        # the attention stream: y += oTp[t] @ Wo[t] (bf16)
                        for lc in range(NQC):
                            py = scr.tile([128, E], F32, name=f"py{t}_{lc}",
                                          tag="pp", bufs=2)
                            nc.tensor.matmul(
                                py[:], oTp[t][:, lc * 128:(lc + 1) * 128],
                                wo_t[t][:], start=True, stop=True)
                            if t == 0:
                                nc.vector.tensor_add(yac[lc][:], py[:],
                                                     bob[:])
                            else:
                                nc.vector.tensor_add(yac[lc][:], yac[lc][:],
                                                     py[:])
                            if t == NT - 1:
                                nc.sync.dma_start(
                                    out[lc * 128:(lc + 1) * 128, :],
                                    yac[lc][:])
                scr_cm.__exit__(None, None, None)

        if reps == 1:
            body()
        else:
            ET = mybir.EngineType
            with tc.For_i(0, reps, 1,
                          hint_engines=(ET.PE, ET.Activation, ET.DVE,
                                        ET.Pool, ET.SP)):
                body()
    with _pin_act_tables():
        nc.compile()
    return nc


def _get_nc(reps=1):
    key = f"nc{reps}"
    if key not in _CACHE:
        _CACHE[key] = _build(reps)
    return _CACHE[key]


def _in_maps(query, key, value, Wq, bq, Wk, bk, Wv, bv, Wo, bo):
    maps = []
    for c in range(N_CORES):
        b, half = c // 2, c % 2
        ls = slice(half * LH, (half + 1) * LH)
        maps.append({
            "src_q": np.ascontiguousarray(query[b, ls]),
            "src_k": np.ascontiguousarray(key[b]),
            "src_v": np.ascontiguousarray(value[b]),
            "wq": np.ascontiguousarray(Wq),
            "wk": np.ascontiguousarray(Wk),
            "wv": np.ascontiguousarray(Wv),
            "wo": np.ascontiguousarray(Wo),
            "bq": np.ascontiguousarray(bq).reshape(1, E),
            "bk": np.ascontiguousarray(bk).reshape(1, E),
            "bv": np.ascontiguousarray(bv).reshape(1, E),
            "bo": np.ascontiguousarray(bo).reshape(1, E),
        })
    return maps


def kernel(**inputs):
    inputs = {k: np.asarray(v, dtype=np.float32) for k, v in inputs.items()}
    nc = _get_nc()
    maps = _in_maps(**inputs)
    res = run_bass_kernel_spmd(nc, maps, list(range(N_CORES)))
    out = np.empty((B, S, E), dtype=np.float32)
    for c in range(N_CORES):
        b, half = c // 2, c % 2
        out[b, half * LH:(half + 1) * LH] = res.results[c]["out"]
    _CACHE["last_maps"] = maps
    return out


def _timed_fn(reps):
    """Jitted sharded single-call executable with device-resident buffers."""
    import jax
    from jax.sharding import Mesh, PartitionSpec, NamedSharding
    from jax.experimental.shard_map import shard_map
    from concourse.bass2jax import (_bass_exec_p, partition_id_tensor,
                                    install_neuronx_cc_hook)

    nc = _get_nc(reps)
    install_neuronx_cc_hook()
    in_names, out_names, out_avals = [], [], []
    for alloc in nc.m.functions[0].allocations:
        if not isinstance(alloc, mybir.MemoryLocationSet):
            continue
        name = alloc.memorylocations[0].name
        if alloc.kind == "ExternalInput":
            if name != "partition_id":
                in_names.append(name)
        elif alloc.kind == "ExternalOutput":
            out_names.append(name)
            out_avals.append(jax.core.ShapedArray(
                tuple(alloc.tensor_shape), mybir.dt.np(alloc.dtype)))
    n_params, n_outs = len(in_names), len(out_names)
    all_in = in_names + out_names + ["partition_id"]

    def _body(*args):
        outs = _bass_exec_p.bind(
            *args, partition_id_tensor(),
            out_avals=tuple(out_avals), in_names=tuple(all_in),
            out_names=tuple(out_names), lowering_input_output_aliases=(),
            sim_require_finite=True, sim_require_nnan=True, nc=nc)
        return tuple(outs)

    devices = jax.devices()[:N_CORES]
    mesh = Mesh(np.asarray(devices), ("core",))
    sh = NamedSharding(mesh, PartitionSpec("core"))
    fn = jax.jit(
        shard_map(_body, mesh=mesh,
                  in_specs=(PartitionSpec("core"),) * (n_params + n_outs),
                  out_specs=(PartitionSpec("core"),) * n_outs,
                  check_rep=False),
        keep_unused=True)
    maps = _CACHE["last_maps"]
    darg = [jax.device_put(
                np.concatenate([np.asarray(maps[c][n]) for c in range(N_CORES)],
                               axis=0), sh) for n in in_names]
    darg += [jax.device_put(
                np.zeros((N_CORES * a.shape[0], *a.shape[1:]), a.dtype), sh)
             for a in out_avals]

    def call():
        import jax as _j
        return _j.block_until_ready(fn(*darg))

    return call


def measure_exec_time_ns(reps=128, trials=28):
    """Per-iteration HW time via in-NEFF hardware-loop repetition delta."""
    import time
    call1 = _timed_fn(1)
    callN = _timed_fn(reps)
    call1(); callN()  # warm both executables

    def timed(call):
        t0 = time.perf_counter()
        call()
        return time.perf_counter() - t0

    deltas = []
    for _ in range(trials):
        t1 = timed(call1)
        tN = timed(callN)
        deltas.append(tN - t1)
    deltas.sort()
    n = len(deltas)
    med = deltas[n // 2] if n % 2 else 0.5 * (deltas[n // 2 - 1]
                                             + deltas[n // 2])
    return int(med / (reps - 1) * 1e9)


if __name__ == "__main__":
    nc = _get_nc()
    print("built + compiled ok")
